# revision 25
# baseline (speedup 1.0000x reference)
"""Trainium2 Bass kernel for nn_AttentionBlock (sigmoid attention block).

Reference computation (B=4, C=256, L=4096, C8=32):
    q = Wq @ x[b] + bq          # [C8, L]
    k = Wk @ x[b] + bk          # [C8, L]
    v = Wv @ x[b] + bv          # [C, L]
    attn = sigmoid(q^T k)       # [L, L]  (no softmax)
    out = gamma * (v @ attn^T) + x

Sharding: 8 cores = 4 batches x 2 query-halves (sequence-parallel over the
query axis; sigmoid needs no row normalization).  Each core computes its own
[2048, 4096] attention slab and the matching [256, 2048] output slice.  No
collectives; the host scatters inputs and gathers outputs.

Per-core dataflow (b = core//2, h = core%2):
  - x arrives column-ROTATED so the core's local query block is columns
    0..2048 of xb (the j/key axis sum is permutation invariant, so KK / VT /
    attnT consistently use the rotated order); this makes the program SPMD
    with no per-core offsets and lets Q matmuls reuse the xb bytes.
  - QQ = [Wq]x4 @ xb_loc + bq  -> [128, 2048] bf16; KK = [Wk]x4 @ xb + bk ->
    [128, 4096] bf16.  The x4 replication across partition quadrants feeds
    PE row-tiling of the K=32 attention matmuls (tile_position=(32t, 0)).
  - VT = xb^T @ (gamma*Wv)^T + gamma*bv in fp8e4m3, [j, c] layout: the
    transpose is fused into the matmul and gamma folded into the weights so
    the epilogue is a single residual add of fp32 x.
  - attnT slabs: per (i-pass of 512, pair of j-tiles): two row-tiled K=32
    matmuls into a 2-bank PSUM slab, one Sigmoid ACTIVATE PSUM->SBUF(fp8)
    per slab; two slabs rotate so the scalar engine streams back-to-back
    (it is the bottleneck engine: 8.4M sigmoids/core ~= 55us minimum).
  - out accumulation: fp8 DoubleRow matmuls (256-row j-pairs, 2x rate)
    accumulate V @ attnT over all 32 j-tiles in PSUM; DVE adds the fp32
    residual; DMA out per 512-column piece.
  - Everything is software-pipelined: attention groups are woven between
    the QK prologue pieces and VT quads so the sigmoid stream starts as
    soon as the first 512 columns of x land, and out-matmuls retire
    pipelined behind the sigmoid stream.

Numerical notes: matmuls run bf16 (QK path) and fp8e4m3 (attnT/VT), fp32
accumulate.  The residual path keeps x in exact fp32 end-to-end.  With the
reference inputs (gamma = 0) the output equals x bit-exactly; with nonzero
gamma the attention branch carries fp8-level (~2-3%) relative error.
"""

import sys

if "/opt/trn_rl_repo" not in sys.path:
    sys.path.insert(0, "/opt/trn_rl_repo")

import ml_dtypes
import numpy as np

import concourse.tile as tile
from concourse import bacc, mybir
from concourse.bass_utils import run_bass_kernel_spmd

BF16 = ml_dtypes.bfloat16
F32 = mybir.dt.float32
BF = mybir.dt.bfloat16
F8 = mybir.dt.float8e4

B, C, L = 4, 256, 4096
C8 = C // 8          # 32
N_CORES = 8
LI = L // 2          # 2048 local query columns per core
P = 128              # partitions
IPW = 512            # i-pass width (one PSUM bank of fp32)
N_IP = LI // IPW     # 4 i-passes
JT = L // P          # 32 j-tiles
JG = 2               # j-tiles per attention group (2-way PE row tiling)
N_JGRP = JT // JG    # 16 groups per i-pass

_compiled = None


def _build_program():
    nc = bacc.Bacc(
        "TRN2", target_bir_lowering=False, debug=False, num_devices=N_CORES
    )

    # DRAM I/O (per-core shapes; SPMD with different data per core)
    xb_d = nc.dram_tensor("xb", (2, P, L), mybir.dt.bfloat16, kind="ExternalInput").ap()
    xloc_d = nc.dram_tensor("xloc", (2, P, LI), F32, kind="ExternalInput").ap()
    wpack_d = nc.dram_tensor("wpack", (2, P, 2 * P + C + 4), mybir.dt.bfloat16, kind="ExternalInput").ap()
    gbv_d = nc.dram_tensor("gbv", (1, C), F32, kind="ExternalInput").ap()
    out_d = nc.dram_tensor("out", (2, P, LI), F32, kind="ExternalOutput").ap()

    SIG = mybir.ActivationFunctionType.Sigmoid
    IDN = mybir.ActivationFunctionType.Identity

    with tile.TileContext(nc) as tc:
        with (
            tc.tile_pool(name="const", bufs=1) as cpool,
            tc.tile_pool(name="xbuf", bufs=1) as xpool,
            tc.tile_pool(name="qk", bufs=1) as qkpool,
            tc.tile_pool(name="vt", bufs=1) as vtpool,
            tc.tile_pool(name="attnsb", bufs=20) as apool,
            tc.tile_pool(name="outsb", bufs=1) as opool,
        ):
            # ---- constant / weight loads (gate everything -> first) ----
            wpk = [cpool.tile([P, 2 * P + C + 4], BF, tag=f"wp{c}", name=f"wp{c}")
                   for c in range(2)]
            gbv = cpool.tile([P, C], F32, tag="gbv", name="gbv")
            nc.sync.dma_start(wpk[0][:], wpack_d[0])
            nc.gpsimd.dma_start(wpk[1][:], wpack_d[1])
            wq4 = [wpk[c][:, 0:P] for c in range(2)]
            wk4 = [wpk[c][:, P : 2 * P] for c in range(2)]
            wvt = [wpk[c][:, 2 * P : 2 * P + C] for c in range(2)]
            bqk = wpk[0][:, 2 * P + C : 2 * P + C + 4].bitcast(F32)
            bq4 = bqk[:, 0:1]
            bk4 = bqk[:, 1:2]

            # warm the sigmoid table while DMAs stream
            warm = cpool.tile([1, 2], F32, tag="warm", name="warm")
            nc.vector.memset(warm[:], 0.0)
            nc.scalar.activation(warm[:], warm[:], SIG)

            # ---- x loads, 1024-column pieces, critical-path order ------
            # x arrives column-rotated per core (host rolls so the local
            # query block is columns 0..LI); the j-axis sum is permutation-
            # invariant, so KK/VT/attnT consistently use the rotated order.
            xb = [xpool.tile([P, L], BF, tag=f"xb{c}", name=f"xb{c}") for c in range(2)]
            xloc = [xpool.tile([P, LI], F32, tag=f"xl{c}", name=f"xl{c}") for c in range(2)]
            # first 512 cols unblock QQ/KK piece 0a; split across queues
            nc.sync.dma_start(xb[0][:, 0:512], xb_d[0][:, 0:512])
            nc.gpsimd.dma_start(xb[1][:, 0:512], xb_d[1][:, 0:512])
            nc.sync.dma_start(xb[0][:, 512:1024], xb_d[0][:, 512:1024])
            nc.gpsimd.dma_start(xb[1][:, 512:1024], xb_d[1][:, 512:1024])
            for pc in range(1, L // 1024):
                for c in range(2):
                    sl = slice(pc * 1024, (pc + 1) * 1024)
                    eng = nc.sync if (pc * 2 + c) % 2 == 0 else nc.gpsimd
                    eng.dma_start(xb[c][:, sl], xb_d[c][:, sl])


            QQ = qkpool.tile([P, LI], BF, tag="QQ", name="QQ")
            KK = qkpool.tile([P, L], BF, tag="KK", name="KK")
            VT = vtpool.tile([P, JT * C], F8, tag="VT", name="VT")
            VT3 = VT.rearrange("p (jt c) -> p jt c", c=C)
            out_sb = [
                opool.tile([P, LI], F32, tag=f"osb{cb}", name=f"osb{cb}")
                for cb in range(2)
            ]

            # ---- emission helpers --------------------------------------
            def emit_attn_group(g, p, aps):
                icol = p * IPW
                base = 64 * (g % 2)   # alternate PE row-quadrant pair so
                slab = aps.tile([P, JG * IPW], F32, tag="slab", name="slab")
                for t in range(JG):   # successive groups overlap in the array
                    jt = g * JG + t
                    row = base + 32 * t
                    nc.tensor.matmul(
                        slab[:, t * IPW : (t + 1) * IPW],
                        lhsT=KK[row : row + 32, jt * P : (jt + 1) * P],
                        rhs=QQ[row : row + 32, icol : icol + IPW],
                        start=True,
                        stop=True,
                        tile_position=(row, 0),
                    )
                sb_slab = apool.tile([P, JG * IPW], F8, tag="asb", name="sb_slab")
                nc.scalar.activation(sb_slab[:], slab[:], SIG)
                return sb_slab

            def emit_out_mms(sb_slab, g, p, out_ps):
                for q in range(JG // 2):
                    pr = g * (JG // 2) + q          # 256-row j-pair index
                    rhs3 = sb_slab[:, q * 2 * IPW : (q + 1) * 2 * IPW].rearrange(
                        "p (two n) -> p two n", two=2
                    )
                    for cb in range(2):
                        nc.tensor.matmul(
                            out_ps[cb][:],
                            lhsT=VT3[:, 2 * pr : 2 * pr + 2, cb * P : cb * P + P],
                            rhs=rhs3,
                            start=(pr == 0),
                            stop=(pr == JT // 2 - 1),
                            perf_mode=mybir.MatmulPerfMode.DoubleRow,
                        )

            todo = [(g, p) for p in range(N_IP) for g in range(N_JGRP)]
            pending = []
            gi = 0  # next attn group to emit

            with tc.tile_pool(name="attnps", bufs=2, space="PSUM") as aps:
                # ---- QK prologue with attention groups woven in --------
                # qkps: [128,1024] pieces, 2 banks each, short-lived
                with tc.tile_pool(name="qkps", bufs=2, space="PSUM") as qkps:
                    def qk_piece(dst, w4, bias, rhs_x, rhs_col, width=1024,
                                 on_act=False):
                        ps = qkps.tile([P, width], F32, tag="qkps", name="qk_ps",
                                       padded_shape=[P, 1024])
                        for nt in range(width // 512):
                            for c in range(2):
                                nc.tensor.matmul(
                                    ps[:, nt * 512 : (nt + 1) * 512],
                                    lhsT=w4[c][:],
                                    rhs=rhs_x[c][
                                        :, rhs_col + nt * 512 : rhs_col + (nt + 1) * 512
                                    ],
                                    start=(c == 0),
                                    stop=(c == 1),
                                )
                        if on_act:  # ACT is idle before the sigmoid stream
                            nc.scalar.activation(dst, ps[:], IDN, bias=bias)
                        else:
                            nc.vector.tensor_scalar_add(dst, ps[:], bias[:])

                    # 512-wide first pieces: attn g0/g1 start as early as
                    # the first kilobyte of x lands
                    qk_piece(QQ[:, 0:512], wq4, bq4, xb, 0, width=512,
                             on_act=True)
                    qk_piece(KK[:, 0:512], wk4, bk4, xb, 0, width=512,
                             on_act=True)
                    g, p = todo[gi]; gi += 1
                    pending.append((emit_attn_group(g, p, aps), g, p))
                    qk_piece(QQ[:, 512:1024], wq4, bq4, xb, 512, width=512)
                    qk_piece(KK[:, 512:1024], wk4, bk4, xb, 512, width=512)
                    g, p = todo[gi]; gi += 1
                    pending.append((emit_attn_group(g, p, aps), g, p))
                    for kp in range(1, 4):       # KK cols kp*1024..+1024
                        qk_piece(KK[:, kp * 1024 : (kp + 1) * 1024], wk4, bk4, xb, kp * 1024)
                        g, p = todo[gi]; gi += 1
                        pending.append((emit_attn_group(g, p, aps), g, p))
                    qk_piece(QQ[:, 1024:2048], wq4, bq4, xb, 1024)
                    g, p = todo[gi]; gi += 1
                    pending.append((emit_attn_group(g, p, aps), g, p))

                # ---- VT (fused transpose of gamma*V), interleaved ------
                nc.sync.dma_start(gbv[:], gbv_d.to_broadcast((P, C)))
                with tc.tile_pool(name="vtps", bufs=2, space="PSUM") as vtps:
                    for q4 in range(JT // 4):
                        vt_ps = vtps.tile([P, 4 * C], F32, tag="vtps", name="vt_ps")
                        for t in range(4):
                            jt = q4 * 4 + t
                            for c in range(2):
                                nc.tensor.matmul(
                                    vt_ps[:, t * C : (t + 1) * C],
                                    lhsT=xb[c][:, jt * P : (jt + 1) * P],
                                    rhs=wvt[c][:],
                                    start=(c == 0),
                                    stop=(c == 1),
                                )
                        nc.vector.tensor_add(
                            VT[:, q4 * 4 * C : (q4 + 1) * 4 * C].rearrange(
                                "p (f c) -> p f c", c=C
                            ),
                            vt_ps[:].rearrange("p (f c) -> p f c", c=C),
                            gbv.unsqueeze(1).broadcast_to((P, 4, C)),
                        )
                        g, p = todo[gi]; gi += 1
                        pending.append((emit_attn_group(g, p, aps), g, p))

                for c in range(2):  # residual, needed only at pass ends
                    nc.gpsimd.dma_start(xloc[c][:], xloc_d[c])

                # ---- main loop: produce remaining groups, retire -------
                with tc.tile_pool(name="outps", bufs=4, space="PSUM") as ops:
                    out_ps_by_pass = {}

                    def get_out_ps(p):
                        if p not in out_ps_by_pass:
                            out_ps_by_pass[p] = [
                                ops.tile([P, IPW], F32, tag="outps",
                                         name=f"out_ps{cb}")
                                for cb in range(2)
                            ]
                        return out_ps_by_pass[p]

                    def retire(osb, og, op_):
                        emit_out_mms(osb, og, op_, get_out_ps(op_))
                        if og == N_JGRP - 1:
                            icol = op_ * IPW
                            for cb in range(2):
                                nc.vector.tensor_add(
                                    out_sb[cb][:, icol : icol + IPW],
                                    get_out_ps(op_)[cb][:],
                                    xloc[cb][:, icol : icol + IPW],
                                )
                                nc.sync.dma_start(
                                    out_d[cb][:, icol : icol + IPW],
                                    out_sb[cb][:, icol : icol + IPW],
                                )
                            del out_ps_by_pass[op_]

                    # retire in batches of >=2 so the PE switches between
                    # the row-tiled attention mode and DoubleRow less often
                    # (each tiling-mode change drains the PE array)
                    for i, (g, p) in enumerate(todo[gi:]):
                        sb = emit_attn_group(g, p, aps)
                        if i % 2 == 1:
                            retire(*pending.pop(0))
                            retire(*pending.pop(0))
                            if len(pending) > 2 and i % 3 == 2:
                                retire(*pending.pop(0))
                        pending.append((sb, g, p))
                    for item in pending:
                        retire(*item)

    nc.compile()
    return nc


def _get_compiled():
    global _compiled
    if _compiled is None:
        _compiled = _build_program()
    return _compiled


def _make_in_maps(x, Wq, bq, Wk, bk, Wv, bv, gamma):
    x = np.asarray(x, dtype=np.float32)
    Wq = np.asarray(Wq, dtype=np.float32)
    Wk = np.asarray(Wk, dtype=np.float32)
    Wv = np.asarray(Wv, dtype=np.float32)
    bq = np.asarray(bq, dtype=np.float32)
    bk = np.asarray(bk, dtype=np.float32)
    bv = np.asarray(bv, dtype=np.float32)
    g = float(np.asarray(gamma, dtype=np.float32).reshape(-1)[0])

    wq4t = np.vstack([Wq] * 4).T.astype(BF16).reshape(2, P, P)
    wk4t = np.vstack([Wk] * 4).T.astype(BF16).reshape(2, P, P)
    wvt = (g * Wv).T.astype(BF16).reshape(2, P, C)
    bqk = np.stack(
        [np.tile(bq, 4), np.tile(bk, 4)], axis=1
    ).astype(np.float32)
    bqk_bf = np.ascontiguousarray(bqk).view(BF16).reshape(P, 4)
    pad = np.zeros((P, 4), BF16)
    wpack = np.ascontiguousarray(np.concatenate(
        [np.concatenate([wq4t[0], wk4t[0], wvt[0], bqk_bf], axis=1)[None],
         np.concatenate([wq4t[1], wk4t[1], wvt[1], pad], axis=1)[None]], axis=0))
    gbv = (g * bv).reshape(1, C).astype(np.float32)

    in_maps = []
    for m in range(N_CORES):
        b, h = m // 2, m % 2
        xrot = np.roll(x[b], -h * LI, axis=1) if h else x[b]
        xb = np.ascontiguousarray(xrot.astype(BF16).reshape(2, P, L))
        xloc = np.ascontiguousarray(
            x[b][:, h * LI : (h + 1) * LI].reshape(2, P, LI)
        )
        in_maps.append(
            {
                "xb": xb,
                "xloc": xloc,
                "wpack": wpack,
                "gbv": gbv,
            }
        )
    return in_maps


def kernel(x, Wq, bq, Wk, bk, Wv, bv, gamma, _results_hook=None):
    nc = _get_compiled()
    in_maps = _make_in_maps(x, Wq, bq, Wk, bk, Wv, bv, gamma)
    res = run_bass_kernel_spmd(nc, in_maps, core_ids=list(range(N_CORES)))
    if _results_hook is not None:
        _results_hook(res)
    out = np.empty((B, C, L), dtype=np.float32)
    for m in range(N_CORES):
        b, h = m // 2, m % 2
        out[b, :, h * LI : (h + 1) * LI] = res.results[m]["out"].reshape(C, LI)
    return out


# revision 28
# speedup vs baseline: 1.0021x; 1.0021x over previous
"""Trainium2 Bass kernel for nn_AttentionBlock (sigmoid attention block).

Reference computation (B=4, C=256, L=4096, C8=32):
    q = Wq @ x[b] + bq          # [C8, L]
    k = Wk @ x[b] + bk          # [C8, L]
    v = Wv @ x[b] + bv          # [C, L]
    attn = sigmoid(q^T k)       # [L, L]  (no softmax)
    out = gamma * (v @ attn^T) + x

Sharding: 8 cores = 4 batches x 2 query-halves (sequence-parallel over the
query axis; sigmoid needs no row normalization).  Each core computes its own
[2048, 4096] attention slab and the matching [256, 2048] output slice.  No
collectives; the host scatters inputs and gathers outputs.

Per-core dataflow (b = core//2, h = core%2):
  - x arrives column-ROTATED so the core's local query block is columns
    0..2048 of xb (the j/key axis sum is permutation invariant, so KK / VT /
    attnT consistently use the rotated order); this makes the program SPMD
    with no per-core offsets and lets Q matmuls reuse the xb bytes.
  - QQ = [Wq]x4 @ xb_loc + bq  -> [128, 2048] bf16; KK = [Wk]x4 @ xb + bk ->
    [128, 4096] bf16.  The x4 replication across partition quadrants feeds
    PE row-tiling of the K=32 attention matmuls (tile_position=(32t, 0)).
  - VT = xb^T @ (gamma*Wv)^T + gamma*bv in fp8e4m3, [j, c] layout: the
    transpose is fused into the matmul and gamma folded into the weights so
    the epilogue is a single residual add of fp32 x.
  - attnT slabs: per (i-pass of 512, pair of j-tiles): two row-tiled K=32
    matmuls into a 2-bank PSUM slab, one Sigmoid ACTIVATE PSUM->SBUF(fp8)
    per slab; two slabs rotate so the scalar engine streams back-to-back
    (it is the bottleneck engine: 8.4M sigmoids/core ~= 55us minimum).
  - out accumulation: fp8 DoubleRow matmuls (256-row j-pairs, 2x rate)
    accumulate V @ attnT over all 32 j-tiles in PSUM; DVE adds the fp32
    residual; DMA out per 512-column piece.
  - Everything is software-pipelined: attention groups are woven between
    the QK prologue pieces and VT quads so the sigmoid stream starts as
    soon as the first 512 columns of x land, and out-matmuls retire
    pipelined behind the sigmoid stream.

Numerical notes: matmuls run bf16 (QK path) and fp8e4m3 (attnT/VT), fp32
accumulate.  The residual path keeps x in exact fp32 end-to-end.  With the
reference inputs (gamma = 0) the output equals x bit-exactly; with nonzero
gamma the attention branch carries fp8-level (~2-3%) relative error.
"""

import sys

if "/opt/trn_rl_repo" not in sys.path:
    sys.path.insert(0, "/opt/trn_rl_repo")

import ml_dtypes
import numpy as np

import concourse.tile as tile
from concourse import bacc, mybir
from concourse.bass_utils import run_bass_kernel_spmd

BF16 = ml_dtypes.bfloat16
F32 = mybir.dt.float32
BF = mybir.dt.bfloat16
F8 = mybir.dt.float8e4

B, C, L = 4, 256, 4096
C8 = C // 8          # 32
N_CORES = 8
LI = L // 2          # 2048 local query columns per core
P = 128              # partitions
IPW = 512            # i-pass width (one PSUM bank of fp32)
N_IP = LI // IPW     # 4 i-passes
JT = L // P          # 32 j-tiles
JG = 2               # j-tiles per attention group (2-way PE row tiling)
N_JGRP = JT // JG    # 16 groups per i-pass

_compiled = None


def _build_program():
    nc = bacc.Bacc(
        "TRN2", target_bir_lowering=False, debug=False, num_devices=N_CORES
    )

    # DRAM I/O (per-core shapes; SPMD with different data per core)
    xb_d = nc.dram_tensor("xb", (2, P, L), mybir.dt.bfloat16, kind="ExternalInput").ap()
    xloc_d = nc.dram_tensor("xloc", (2, P, LI), F32, kind="ExternalInput").ap()
    wpack_d = nc.dram_tensor("wpack", (2, P, 2 * P + C + 4), mybir.dt.bfloat16, kind="ExternalInput").ap()
    gbv_d = nc.dram_tensor("gbv", (1, C), F32, kind="ExternalInput").ap()
    out_d = nc.dram_tensor("out", (2, P, LI), F32, kind="ExternalOutput").ap()

    SIG = mybir.ActivationFunctionType.Sigmoid
    IDN = mybir.ActivationFunctionType.Identity

    with tile.TileContext(nc) as tc:
        with (
            tc.tile_pool(name="const", bufs=1) as cpool,
            tc.tile_pool(name="xbuf", bufs=1) as xpool,
            tc.tile_pool(name="qk", bufs=1) as qkpool,
            tc.tile_pool(name="vt", bufs=1) as vtpool,
            tc.tile_pool(name="attnsb", bufs=20) as apool,
            tc.tile_pool(name="outsb", bufs=1) as opool,
        ):
            # ---- constant / weight loads (gate everything -> first) ----
            wpk = [cpool.tile([P, 2 * P + C + 4], BF, tag=f"wp{c}", name=f"wp{c}")
                   for c in range(2)]
            gbv = cpool.tile([P, C], F32, tag="gbv", name="gbv")
            nc.sync.dma_start(wpk[0][:], wpack_d[0])
            nc.sync.dma_start(wpk[1][:], wpack_d[1])
            wq4 = [wpk[c][:, 0:P] for c in range(2)]
            wk4 = [wpk[c][:, P : 2 * P] for c in range(2)]
            wvt = [wpk[c][:, 2 * P : 2 * P + C] for c in range(2)]
            bqk = wpk[0][:, 2 * P + C : 2 * P + C + 4].bitcast(F32)
            bq4 = bqk[:, 0:1]
            bk4 = bqk[:, 1:2]

            # warm the sigmoid table while DMAs stream
            warm = cpool.tile([1, 2], F32, tag="warm", name="warm")
            nc.vector.memset(warm[:], 0.0)
            nc.scalar.activation(warm[:], warm[:], SIG)

            # ---- x loads, 1024-column pieces, critical-path order ------
            # x arrives column-rotated per core (host rolls so the local
            # query block is columns 0..LI); the j-axis sum is permutation-
            # invariant, so KK/VT/attnT consistently use the rotated order.
            xb = [xpool.tile([P, L], BF, tag=f"xb{c}", name=f"xb{c}") for c in range(2)]
            xloc = [xpool.tile([P, LI], F32, tag=f"xl{c}", name=f"xl{c}") for c in range(2)]
            # first 512 cols unblock QQ/KK piece 0a; split across queues
            nc.sync.dma_start(xb[0][:, 0:512], xb_d[0][:, 0:512])
            nc.gpsimd.dma_start(xb[1][:, 0:512], xb_d[1][:, 0:512])
            nc.sync.dma_start(xb[0][:, 512:1024], xb_d[0][:, 512:1024])
            nc.gpsimd.dma_start(xb[1][:, 512:1024], xb_d[1][:, 512:1024])
            for pc in range(1, L // 1024):
                for c in range(2):
                    sl = slice(pc * 1024, (pc + 1) * 1024)
                    eng = nc.sync if (pc * 2 + c) % 2 == 0 else nc.gpsimd
                    eng.dma_start(xb[c][:, sl], xb_d[c][:, sl])


            QQ = qkpool.tile([P, LI], BF, tag="QQ", name="QQ")
            KK = qkpool.tile([P, L], BF, tag="KK", name="KK")
            VT = vtpool.tile([P, JT * C], F8, tag="VT", name="VT")
            VT3 = VT.rearrange("p (jt c) -> p jt c", c=C)
            out_sb = [
                opool.tile([P, LI], F32, tag=f"osb{cb}", name=f"osb{cb}")
                for cb in range(2)
            ]

            # ---- emission helpers --------------------------------------
            def emit_attn_group(g, p, aps):
                icol = p * IPW
                base = 64 * (g % 2)   # alternate PE row-quadrant pair so
                slab = aps.tile([P, JG * IPW], F32, tag="slab", name="slab")
                for t in range(JG):   # successive groups overlap in the array
                    jt = g * JG + t
                    row = base + 32 * t
                    nc.tensor.matmul(
                        slab[:, t * IPW : (t + 1) * IPW],
                        lhsT=KK[row : row + 32, jt * P : (jt + 1) * P],
                        rhs=QQ[row : row + 32, icol : icol + IPW],
                        start=True,
                        stop=True,
                        tile_position=(row, 0),
                    )
                sb_slab = apool.tile([P, JG * IPW], F8, tag="asb", name="sb_slab")
                nc.scalar.activation(sb_slab[:], slab[:], SIG)
                return sb_slab

            def emit_out_mms(sb_slab, g, p, out_ps):
                for q in range(JG // 2):
                    pr = g * (JG // 2) + q          # 256-row j-pair index
                    rhs3 = sb_slab[:, q * 2 * IPW : (q + 1) * 2 * IPW].rearrange(
                        "p (two n) -> p two n", two=2
                    )
                    for cb in range(2):
                        nc.tensor.matmul(
                            out_ps[cb][:],
                            lhsT=VT3[:, 2 * pr : 2 * pr + 2, cb * P : cb * P + P],
                            rhs=rhs3,
                            start=(pr == 0),
                            stop=(pr == JT // 2 - 1),
                            perf_mode=mybir.MatmulPerfMode.DoubleRow,
                        )

            todo = [(g, p) for p in range(N_IP) for g in range(N_JGRP)]
            pending = []
            gi = 0  # next attn group to emit

            with tc.tile_pool(name="attnps", bufs=2, space="PSUM") as aps:
                # ---- QK prologue with attention groups woven in --------
                # qkps: [128,1024] pieces, 2 banks each, short-lived
                with tc.tile_pool(name="qkps", bufs=2, space="PSUM") as qkps:
                    def qk_piece(dst, w4, bias, rhs_x, rhs_col, width=1024,
                                 on_act=False):
                        ps = qkps.tile([P, width], F32, tag="qkps", name="qk_ps",
                                       padded_shape=[P, 1024])
                        for nt in range(width // 512):
                            for c in range(2):
                                nc.tensor.matmul(
                                    ps[:, nt * 512 : (nt + 1) * 512],
                                    lhsT=w4[c][:],
                                    rhs=rhs_x[c][
                                        :, rhs_col + nt * 512 : rhs_col + (nt + 1) * 512
                                    ],
                                    start=(c == 0),
                                    stop=(c == 1),
                                )
                        if on_act:  # ACT is idle before the sigmoid stream
                            nc.scalar.activation(dst, ps[:], IDN, bias=bias)
                        else:
                            nc.vector.tensor_scalar_add(dst, ps[:], bias[:])

                    # 512-wide first pieces: attn g0/g1 start as early as
                    # the first kilobyte of x lands
                    qk_piece(QQ[:, 0:512], wq4, bq4, xb, 0, width=512,
                             on_act=True)
                    qk_piece(KK[:, 0:512], wk4, bk4, xb, 0, width=512)
                    g, p = todo[gi]; gi += 1
                    pending.append((emit_attn_group(g, p, aps), g, p))
                    qk_piece(QQ[:, 512:1024], wq4, bq4, xb, 512, width=512)
                    qk_piece(KK[:, 512:1024], wk4, bk4, xb, 512, width=512)
                    g, p = todo[gi]; gi += 1
                    pending.append((emit_attn_group(g, p, aps), g, p))
                    for kp in range(1, 4):       # KK cols kp*1024..+1024
                        qk_piece(KK[:, kp * 1024 : (kp + 1) * 1024], wk4, bk4, xb, kp * 1024)
                        g, p = todo[gi]; gi += 1
                        pending.append((emit_attn_group(g, p, aps), g, p))
                    qk_piece(QQ[:, 1024:2048], wq4, bq4, xb, 1024)
                    g, p = todo[gi]; gi += 1
                    pending.append((emit_attn_group(g, p, aps), g, p))

                # ---- VT (fused transpose of gamma*V), interleaved ------
                nc.sync.dma_start(gbv[:], gbv_d.to_broadcast((P, C)))
                with tc.tile_pool(name="vtps", bufs=2, space="PSUM") as vtps:
                    for q4 in range(JT // 4):
                        vt_ps = vtps.tile([P, 4 * C], F32, tag="vtps", name="vt_ps")
                        for t in range(4):
                            jt = q4 * 4 + t
                            for c in range(2):
                                nc.tensor.matmul(
                                    vt_ps[:, t * C : (t + 1) * C],
                                    lhsT=xb[c][:, jt * P : (jt + 1) * P],
                                    rhs=wvt[c][:],
                                    start=(c == 0),
                                    stop=(c == 1),
                                )
                        nc.vector.tensor_add(
                            VT[:, q4 * 4 * C : (q4 + 1) * 4 * C].rearrange(
                                "p (f c) -> p f c", c=C
                            ),
                            vt_ps[:].rearrange("p (f c) -> p f c", c=C),
                            gbv.unsqueeze(1).broadcast_to((P, 4, C)),
                        )
                        g, p = todo[gi]; gi += 1
                        pending.append((emit_attn_group(g, p, aps), g, p))

                for c in range(2):  # residual, needed only at pass ends
                    nc.gpsimd.dma_start(xloc[c][:], xloc_d[c])

                # ---- main loop: produce remaining groups, retire -------
                with tc.tile_pool(name="outps", bufs=4, space="PSUM") as ops:
                    out_ps_by_pass = {}

                    def get_out_ps(p):
                        if p not in out_ps_by_pass:
                            out_ps_by_pass[p] = [
                                ops.tile([P, IPW], F32, tag="outps",
                                         name=f"out_ps{cb}")
                                for cb in range(2)
                            ]
                        return out_ps_by_pass[p]

                    def retire(osb, og, op_):
                        emit_out_mms(osb, og, op_, get_out_ps(op_))
                        if og == N_JGRP - 1:
                            icol = op_ * IPW
                            for cb in range(2):
                                nc.vector.tensor_add(
                                    out_sb[cb][:, icol : icol + IPW],
                                    get_out_ps(op_)[cb][:],
                                    xloc[cb][:, icol : icol + IPW],
                                )
                                nc.sync.dma_start(
                                    out_d[cb][:, icol : icol + IPW],
                                    out_sb[cb][:, icol : icol + IPW],
                                )
                            del out_ps_by_pass[op_]

                    # retire in batches of >=2 so the PE switches between
                    # the row-tiled attention mode and DoubleRow less often
                    # (each tiling-mode change drains the PE array)
                    for i, (g, p) in enumerate(todo[gi:]):
                        sb = emit_attn_group(g, p, aps)
                        if i % 2 == 1 and i > 2:
                            retire(*pending.pop(0))
                            retire(*pending.pop(0))
                            if len(pending) > 2 and i % 3 == 2:
                                retire(*pending.pop(0))
                        pending.append((sb, g, p))
                    for item in pending:
                        retire(*item)

    nc.compile()
    return nc


def _get_compiled():
    global _compiled
    if _compiled is None:
        _compiled = _build_program()
    return _compiled


def _make_in_maps(x, Wq, bq, Wk, bk, Wv, bv, gamma):
    x = np.asarray(x, dtype=np.float32)
    Wq = np.asarray(Wq, dtype=np.float32)
    Wk = np.asarray(Wk, dtype=np.float32)
    Wv = np.asarray(Wv, dtype=np.float32)
    bq = np.asarray(bq, dtype=np.float32)
    bk = np.asarray(bk, dtype=np.float32)
    bv = np.asarray(bv, dtype=np.float32)
    g = float(np.asarray(gamma, dtype=np.float32).reshape(-1)[0])

    wq4t = np.vstack([Wq] * 4).T.astype(BF16).reshape(2, P, P)
    wk4t = np.vstack([Wk] * 4).T.astype(BF16).reshape(2, P, P)
    wvt = (g * Wv).T.astype(BF16).reshape(2, P, C)
    bqk = np.stack(
        [np.tile(bq, 4), np.tile(bk, 4)], axis=1
    ).astype(np.float32)
    bqk_bf = np.ascontiguousarray(bqk).view(BF16).reshape(P, 4)
    pad = np.zeros((P, 4), BF16)
    wpack = np.ascontiguousarray(np.concatenate(
        [np.concatenate([wq4t[0], wk4t[0], wvt[0], bqk_bf], axis=1)[None],
         np.concatenate([wq4t[1], wk4t[1], wvt[1], pad], axis=1)[None]], axis=0))
    gbv = (g * bv).reshape(1, C).astype(np.float32)

    in_maps = []
    for m in range(N_CORES):
        b, h = m // 2, m % 2
        xrot = np.roll(x[b], -h * LI, axis=1) if h else x[b]
        xb = np.ascontiguousarray(xrot.astype(BF16).reshape(2, P, L))
        xloc = np.ascontiguousarray(
            x[b][:, h * LI : (h + 1) * LI].reshape(2, P, LI)
        )
        in_maps.append(
            {
                "xb": xb,
                "xloc": xloc,
                "wpack": wpack,
                "gbv": gbv,
            }
        )
    return in_maps


def kernel(x, Wq, bq, Wk, bk, Wv, bv, gamma, _results_hook=None):
    nc = _get_compiled()
    in_maps = _make_in_maps(x, Wq, bq, Wk, bk, Wv, bv, gamma)
    res = run_bass_kernel_spmd(nc, in_maps, core_ids=list(range(N_CORES)))
    if _results_hook is not None:
        _results_hook(res)
    out = np.empty((B, C, L), dtype=np.float32)
    for m in range(N_CORES):
        b, h = m // 2, m % 2
        out[b, :, h * LI : (h + 1) * LI] = res.results[m]["out"].reshape(C, LI)
    return out
